# revision 42
# baseline (speedup 1.0000x reference)
"""Single-head self-attention (B=4, S=2048, D=1024) on 8 Trainium2 NeuronCores.

v5: collective-free via weight folding + reassociation. Core c handles batch
b = c//2 and query half h = c%2 (1024 queries). Two algebraic rewrites kill
all inter-core communication:

  logits L = (x WQ)(x WK)^T / sqrt(D) = x M x^T   with M = WQ WK^T / sqrt(D)
  O = P (x WV) = (P x) WV = (x^T P)^T WV          (G^T := x^T P)

M is folded on the host (weight-only preprocessing, like conv/BN folding), so
the device needs no K projection and no K/V exchange: each core only needs its
batch's full x (shipped in both layouts, x^T for L and x for G^T) plus M and
WV. v3's three serialized pair AllGathers (~58us CC busy, ~20us PE stall
waiting on ccV2-fed data) disappear, as does the 8MB/core gather-reload DMA.
Per-core PE drops from 7 to 6 matmul units = 768 512-wide matmuls ~166us (the
K projection is the unit that M-folding deletes globally; 6 units/core is the
floor for this factorization -- no cross-core redundancy remains to shard).

SPMD uniformity: "own queries" are a h-dependent slice of the sequence, so the
host permutes each core's key/sequence order to own-half-first (softmax and
O are key-permutation invariant as long as x^T and x use the same permutation;
each core writes only its own query rows, un-permuted by the host gather).

Schedule per core (all bf16 operands, fp32 PSUM): memset constants + 160 PE
warmup mms right off the framework preamble (~7.1us); Q'^T = M^T x^T resident
(128 mm) pipelining with the input DMA, whose (m, xt) 512-col pieces land in
exact first-use order split across both rings -- only the qb=0 half runs up
front (group-0 strips need just Q' cols 0:512), with the qb=1 chains
deferred until after those strips so they run compute-paced once the B-half
DMAs have landed. The xs/wv loads ride the SYNC ring: load triggers occupy
the issuing engine queue ~600ns each, and 24 extra triggers on the scalar
queue measured as a 4.4us exp-then-PE stall once the strips start at ~27us.
Per q-group of 512: 16 L
strips (8 mm each) with exp -> bf16 P strips, the rowsum done as 4 free-dim-1
matmuls per strip (P slice stationary -> already-transposed [128,4] PSUM
accumulation; measured ~37ns each interleaved mid-chain -- a 16x216ns
full-width chain variant measured 6us SLOWER end-to-end, and GpSimd
CROSS_LANE_REDUCE is ~64us/strip, 150x too slow); G^T accumulated over the
16 P strips in two passes of 4 banks from the SHARED 7-buf PSUM pool that
also serves the Q' chains, L strips and O chains (identical [128,512]
write->read->free lifetime; one pool means no mid-kernel PSUM pool
transition, which measured as a ~0.8us barrier, and 7-deep rotation slack
everywhere; the 8th bank holds the rowsum accumulator); the kernel's very
last O chain is split into two 256-col sub-chains so its scale+store
pipeline under the second half. Output scaled by reciprocal rowsum straight
to bf16; mid-kernel stores all ride the sync ring (idle after the input
loads, so store triggers never jitter the scalar queue's exps), with only
the final split pair using both rings. fp8/DoubleRow was
measured numerically dead for this problem's 2e-2 gate (exp() amplifies
quant noise into multiplicative P error; every e4m3 insertion point alone
measures ~2e-2+ on the real inputs -- P/V/QK/xW, and even a Q'-phase-only
variant at the best M-scaling measured 2.8e-2), so everything stays bf16.

Verified on HW: 190.9us best / ~191-193us band across clean-clock runs (vs
261.9us v3 baseline; shared-device throttling occasionally inflates samples
to 198-229us with nonzero throttle counters), rel_err 4.29e-3 (gate 2e-2),
matching the numpy simulation of the pipeline exactly. PE busy ~176us of
the ~186.4us first-to-last-matmul span; startup is at the per-core HBM
bandwidth wall (~2MB gating set over 2 rings from the 7.3us ring start),
and the rest is the fixed ~7.4us framework preamble, ~4.7us tail/teardown,
and ~2.2us of metronomic ~432ns PE stalls every ~10.8us (50 matmuls) of
unknown, likely instruction-path, origin.
"""

import numpy as np
from contextlib import ExitStack

import ml_dtypes

import concourse.tile as tile
from concourse import bacc, mybir
from concourse.bass_utils import run_bass_kernel_spmd

F32 = mybir.dt.float32
BF16 = mybir.dt.bfloat16
EXP = mybir.ActivationFunctionType.Exp

B, S, D = 4, 2048, 1024
NQ = 1024          # query rows per core (own half, permuted-first)
QG = 512           # q-group width for the attention passes
NGROUPS = NQ // QG
NET = D // 128     # 8 e'-tiles (Q' feature dim)
NDT = D // 128     # 8 d-tiles (x feature dim)
NKT = S // 128     # 16 key/sequence tiles

_CACHE = {}


def _build_nc():
    nc = bacc.Bacc("TRN2", target_bir_lowering=False, debug=False)

    xt_d = nc.dram_tensor("xt", [D, S], BF16, kind="ExternalInput")
    xs_d = nc.dram_tensor("xs", [S, D], BF16, kind="ExternalInput")
    m_d = nc.dram_tensor("m", [D, D], BF16, kind="ExternalInput")
    wv_d = nc.dram_tensor("wv", [D, D], BF16, kind="ExternalInput")
    o_d = nc.dram_tensor("o", [NQ, D], BF16, kind="ExternalOutput")

    def dslc(i):
        return slice(i * 128, (i + 1) * 128)

    with tile.TileContext(nc) as tc, ExitStack() as ctx:
        # Constants come from on-device memsets (no DMA: the rings' head
        # slots stay free for the m/xt loads, and the PE warmup can start
        # right after the framework preamble instead of waiting on a DMA).
        small = ctx.enter_context(tc.tile_pool(name="small", bufs=1))
        ones16 = small.tile([128, 2], BF16, name="ones16", tag="ones16")
        nc.vector.memset(ones16[:], 1.0)
        ones32 = small.tile([1, 2], F32, name="ones32", tag="ones32")
        nc.vector.memset(ones32[:], 1.0)
        exp_warm = small.tile([1, 2], F32, name="exp_warm", tag="exp_warm")
        nc.scalar.activation(exp_warm[:], ones32[:], EXP, bias=0.0, scale=1.0)

        # M is only needed for the Q' phase: right stack, released before
        # the attention pools are allocated.
        mres = tc.alloc_tile_pool(name="mres", bufs=1, side="right")
        m_sb = [mres.tile([128, D], BF16, name=f"m{dt}", tag=f"m{dt}")
                for dt in range(NDT)]

        # Long-lived residents.
        xtres = ctx.enter_context(tc.tile_pool(name="xtres", bufs=1))
        xt_sb = [xtres.tile([128, S], BF16, name=f"xt{et}", tag=f"xt{et}")
                 for et in range(NET)]
        xsres = ctx.enter_context(tc.tile_pool(name="xsres", bufs=1))
        xs_sb = [xsres.tile([128, D], BF16, name=f"xs{st}", tag=f"xs{st}")
                 for st in range(NKT)]
        qres = ctx.enter_context(tc.tile_pool(name="qres", bufs=1))
        qt_sb = [qres.tile([128, NQ], BF16, name=f"qt{et}", tag=f"qt{et}")
                 for et in range(NET)]
        wvres = ctx.enter_context(tc.tile_pool(name="wvres", bufs=1))
        wv_sb = [wvres.tile([128, D], BF16, name=f"wv{dt}", tag=f"wv{dt}")
                 for dt in range(NDT)]

        # Input DMA split across both rings, ordered by first use. The first
        # Q' chain consumes (m[dt] cols 0:512, xt[dt] cols 0:512) for dt
        # 0..7 in order, so those land as interleaved (m, xt) pairs -- dt
        # 0-3 on the sync ring, dt 4-7 on scalar -- letting the chain's
        # matmuls pipeline with the DMA arrivals instead of waiting for a
        # bulk load. Then the B halves (m cols 512:1024 for the et>=4
        # chains, xt cols 512:1024 for qb=1), then the other-sequence-half
        # xt columns (first used at strip kt=8, ~55us), then xs (G pass,
        # ~70us) and wv (O chains, ~85us). Per-DMA ring occupancy is
        # ~600ns nearly independent of size in this regime, and a
        # full-width [128,1024]-piece variant measured 1.6us slower, so
        # the 512-col granularity stays.
        # DMA triggers past ~11 per ring are throttled by semaphore-slot
        # recycling (each new trigger waits its slot's previous consumers,
        # i.e. chain matmuls at 1.73us pace) -- so every piece the qb=0
        # chains need must sit inside the free-issue window: (m-A, xt-own
        # FULL [0:1024]) pairs at positions 1-8 and all m-B at 9-12 per
        # ring. Loading xt-own full-width also deletes the xt-B DMAs
        # outright. Later loads are deadline-safe even at throttled pace:
        # xt-other (strip kt>=8, ~42us) on scalar (whose queue still drains
        # before the strip exps come due -- extra triggers there once
        # measured as a 4.4us exp stall), xs/wv (>=71us) on sync.
        for dt in range(4):
            nc.sync.dma_start(m_sb[dt][:, 0:512], m_d.ap()[dslc(dt), 0:512])
            nc.sync.dma_start(xt_sb[dt][:, 0:1024],
                              xt_d.ap()[dslc(dt), 0:1024])
        for dt in range(4, NDT):
            nc.scalar.dma_start(m_sb[dt][:, 0:512], m_d.ap()[dslc(dt), 0:512])
            nc.scalar.dma_start(xt_sb[dt][:, 0:1024],
                                xt_d.ap()[dslc(dt), 0:1024])
        for dt in range(4):
            nc.sync.dma_start(m_sb[dt][:, 512:1024],
                              m_d.ap()[dslc(dt), 512:1024])
        for dt in range(4, NDT):
            nc.scalar.dma_start(m_sb[dt][:, 512:1024],
                                m_d.ap()[dslc(dt), 512:1024])
        for dt in range(NDT):
            nc.scalar.dma_start(xt_sb[dt][:, 1024:2048],
                                xt_d.ap()[dslc(dt), 1024:2048])
        for st in range(NKT):
            nc.sync.dma_start(xs_sb[st][:], xs_d.ap()[dslc(st), :])
        for dt in range(NDT):
            nc.sync.dma_start(wv_sb[dt][:], wv_d.ap()[dslc(dt), :])

        # One shared 7-buf PSUM pool for every [128,512] accumulator: Q'
        # chains, L strips, G chains, and O chains all have the same
        # sequential write -> read -> free lifetime, so a single-tag
        # rotation gives every phase 7 banks of lookahead and there are NO
        # mid-kernel pool transitions (a released-then-reallocated PSUM
        # pool was measured to insert a ~0.8us barrier at the phase
        # boundary). The 8th bank holds the long-lived rowsum accumulator.
        bigps = ctx.enter_context(
            tc.tile_pool(name="bigps", bufs=7, space="PSUM"))
        rsps = ctx.enter_context(
            tc.tile_pool(name="rsps", bufs=1, space="PSUM"))

        # PE warmup during the input DMA (keeps HAM ramped into the Q'
        # phase; each warm matmul is ~60ns). The warm target is just the
        # first rotation slot of the shared pool.
        warm_ps = bigps.tile([1, 2], F32, name="warm_ps", tag="ps")
        for _ in range(160):
            nc.tensor.matmul(warm_ps[:], ones16[:, 0:1], ones16[:, 0:2],
                             start=True, stop=True)

        # ---- Q' phase: Q'^T = M^T x^T resident (own queries = cols 0:NQ).
        # Only the qb=0 half runs up front: group-0 strips need just Q'
        # cols 0:512, so the qb=1 chains are deferred until after those
        # strips -- by then the m/xt B-half DMAs have landed with ~30us of
        # slack and the deferred chains run fully compute-paced instead of
        # stalling on the second DMA wave.
        def qprime_chains(qb):
            for et in range(NET):
                ps = bigps.tile([128, 512], F32, name="qp_ps", tag="ps")
                for dt in range(NDT):
                    nc.tensor.matmul(
                        ps[:],
                        m_sb[dt][:, et * 128:(et + 1) * 128],
                        xt_sb[dt][:, qb * 512:(qb + 1) * 512],
                        start=(dt == 0), stop=(dt == NDT - 1))
                nc.vector.tensor_copy(
                    qt_sb[et][:, qb * 512:(qb + 1) * 512], ps[:])

        qprime_chains(0)

        # ---- Attention per q-group: L strips -> exp -> G^T -> O ----
        with tc.tile_pool(name="attp", bufs=2) as attp, \
             tc.tile_pool(name="gsbp", bufs=2) as gsbp, \
             tc.tile_pool(name="rssb", bufs=2) as rssb, \
             tc.tile_pool(name="osbp", bufs=3) as osbp:

            for g in range(NGROUPS):
                qslc = slice(g * QG, (g + 1) * QG)

                # L strips: L[k, q] = sum_e' x^T[e', k] Q'^T[e', q].
                # Rowsum: per strip, 4 free-dim-1 matmuls with the P slice
                # as the STATIONARY operand (out[128q, 1] += P[:, qtl].T @
                # ones) accumulate the rowsum already transposed in a
                # [128, 4] PSUM tile -- no [1,512] accumulator bank, no
                # transpose pass. They lag TWO strips behind the exp and are
                # interleaved mid-chain so their ldweights hide under the
                # 216ns L matmuls. (GpSimd CROSS_LANE_REDUCE was measured at
                # ~64us per [128,512] strip -- 150x too slow to use.)
                rs_t_ps = rsps.tile([128, QG // 128], F32, name="rs_t_ps",
                                    tag="rs_t_ps")

                def rowsum_mm(src_kt, qtl):
                    # start only on the very first matmul: start=True zeroes
                    # the whole 2KB PSUM zero-region (all 4 columns), and
                    # each column's first touch then consumes the pending-
                    # zero marking. A per-column start would wipe sibling
                    # columns' already-accumulated strip-0 values.
                    nc.tensor.matmul(
                        rs_t_ps[:, qtl:qtl + 1],
                        pt_strip[src_kt][:, qtl * 128:(qtl + 1) * 128],
                        ones16[:, 0:1],
                        start=(src_kt == 0 and qtl == 0),
                        stop=(src_kt == NKT - 1 and qtl == QG // 128 - 1))

                pt_strip = []
                for kt in range(NKT):
                    ps = bigps.tile([128, QG], F32, name="st_ps", tag="ps")
                    for et in range(NET):
                        nc.tensor.matmul(
                            ps[:],
                            xt_sb[et][:, kt * 128:(kt + 1) * 128],
                            qt_sb[et][:, qslc],
                            start=(et == 0), stop=(et == NET - 1))
                        if 3 <= et <= 6 and kt >= 2:
                            rowsum_mm(kt - 2, et - 3)
                    pt = attp.tile([128, QG], BF16, name=f"pt{kt}",
                                   tag=f"pt{kt}")
                    nc.scalar.activation(pt[:], ps[:], EXP, bias=0.0,
                                         scale=1.0)
                    pt_strip.append(pt)
                for src_kt in (NKT - 2, NKT - 1):
                    for qtl in range(QG // 128):
                        rowsum_mm(src_kt, qtl)

                # Deferred qb=1 Q' chains: after group-0's strips, before
                # its G passes (their B-half inputs landed ~35us ago).
                if g == 0:
                    qprime_chains(1)
                    mres.release()

                # G^T pass A (d-tiles 0-3): G^T[d, q] = sum_k x[k, d] P[k, q]
                g_sb = [None] * NDT

                def g_chain(dt):
                    gps = bigps.tile([128, QG], F32, name="gt_ps", tag="ps")
                    for kt in range(NKT):
                        nc.tensor.matmul(
                            gps[:],
                            xs_sb[kt][:, dt * 128:(dt + 1) * 128],
                            pt_strip[kt][:],
                            start=(kt == 0), stop=(kt == NKT - 1))
                    g_sb[dt] = gsbp.tile([128, QG], BF16, name=f"gt{dt}",
                                         tag=f"gt{dt}")
                    nc.vector.tensor_copy(g_sb[dt][:], gps[:])

                for dt in range(4):
                    g_chain(dt)

                # Rowsum finalize: the transposed [128, 4] accumulation only
                # needs a DVE reciprocal (first O-chain consumer is ~16us
                # out, well past the last rowsum matmul).
                rs_sb = rssb.tile([128, QG // 128], F32, name="rs_sb",
                                  tag="rs_sb")
                for qtl in range(QG // 128):
                    nc.vector.reciprocal(rs_sb[:, qtl:qtl + 1],
                                         rs_t_ps[:, qtl:qtl + 1])

                # G^T pass B (d-tiles 4-7) reuses the same 4 PSUM bufs; the
                # pass-A casts complete long before rotation needs them.
                for dt in range(4, NDT):
                    g_chain(dt)

                # O chains: O[q, e] = sum_d G^T[d, q]^T WV[d, e], scaled by
                # the reciprocal rowsum, stored bf16 on alternating rings.
                # The very last chain of the kernel is split into two
                # 256-col sub-chains so its scale+store pipeline under the
                # second half (and the final stores ride both rings),
                # trimming the exposed tail after the last matmul.
                for qtl in range(QG // 128):
                    for eb in range(2):
                        last = (g == NGROUPS - 1 and qtl == QG // 128 - 1
                                and eb == 1)
                        ebs = [(eb * 512, 256), (eb * 512 + 256, 256)] \
                            if last else [(eb * 512, 512)]
                        for ci, (c0, cw) in enumerate(ebs):
                            ops = bigps.tile([128, QG], F32, name="o_ps",
                                            tag="ps")
                            for dt in range(NDT):
                                nc.tensor.matmul(
                                    ops[:, 0:cw],
                                    g_sb[dt][:, qtl * 128:(qtl + 1) * 128],
                                    wv_sb[dt][:, c0:c0 + cw],
                                    start=(dt == 0), stop=(dt == NDT - 1))
                            osb = osbp.tile([128, 512], BF16, name="o_sb",
                                            tag="o_sb")
                            nc.vector.tensor_scalar_mul(
                                osb[:, 0:cw], ops[:, 0:cw],
                                rs_sb[:, qtl:qtl + 1])
                            # All mid-kernel stores ride the sync ring (idle
                            # after the input loads) so store triggers never
                            # jitter the scalar queue's exps; only the final
                            # split pair uses both rings for the tail.
                            eng = nc.scalar if (last and ci == 1) else nc.sync
                            eng.dma_start(
                                o_d.ap()[g * QG + qtl * 128:
                                         g * QG + (qtl + 1) * 128,
                                         c0:c0 + cw],
                                osb[:, 0:cw])

    nc.compile()
    return nc


def get_nc():
    if "nc" not in _CACHE:
        _CACHE["nc"] = _build_nc()
    return _CACHE["nc"]


def make_in_maps(x, WQ, WK, WV):
    bf16 = ml_dtypes.bfloat16
    # Weight folding on host (fp64 for accuracy): M = WQ WK^T / sqrt(D).
    M = (np.asarray(WQ, np.float64) @ np.asarray(WK, np.float64).T
         / np.sqrt(float(D)))
    m16 = np.ascontiguousarray(M.astype(np.float32).astype(bf16))
    wv16 = np.ascontiguousarray(np.asarray(WV, np.float32).astype(bf16))
    in_maps = []
    for c in range(8):
        b, h = c // 2, c % 2
        xb = np.asarray(x[b], np.float32)
        # Own-half-first sequence permutation keeps the SPMD program uniform.
        xp = np.concatenate(
            [xb[h * NQ:(h + 1) * NQ], xb[(1 - h) * NQ:(2 - h) * NQ]], axis=0)
        xp16 = xp.astype(bf16)
        in_maps.append({"xt": np.ascontiguousarray(xp16.T),
                        "xs": np.ascontiguousarray(xp16),
                        "m": m16, "wv": wv16})
    return in_maps


def kernel(**inputs):
    x = np.asarray(inputs["x"], dtype=np.float32)
    WQ = np.asarray(inputs["WQ"], dtype=np.float32)
    WK = np.asarray(inputs["WK"], dtype=np.float32)
    WV = np.asarray(inputs["WV"], dtype=np.float32)

    nc = get_nc()
    in_maps = make_in_maps(x, WQ, WK, WV)
    res = run_bass_kernel_spmd(nc, in_maps, core_ids=list(range(8)))

    out = np.empty((B, S, D), np.float32)
    for c in range(8):
        b, h = c // 2, c % 2
        out[b, h * NQ:(h + 1) * NQ, :] = np.asarray(
            res.results[c]["o"], dtype=np.float32)
    return out


if __name__ == "__main__":
    rng = np.random.default_rng(0)
    x = rng.standard_normal((B, S, D), dtype=np.float32)
    WQ = (rng.standard_normal((D, D), dtype=np.float32) * 0.02)
    WK = (rng.standard_normal((D, D), dtype=np.float32) * 0.02)
    WV = (rng.standard_normal((D, D), dtype=np.float32) * 0.02)
    o = kernel(x=x, WQ=WQ, WK=WK, WV=WV)
    print("out", o.shape, o.dtype, float(np.abs(o).max()))


# revision 43
# speedup vs baseline: 1.0077x; 1.0077x over previous
"""Single-head self-attention (B=4, S=2048, D=1024) on 8 Trainium2 NeuronCores.

v5: collective-free via weight folding + reassociation. Core c handles batch
b = c//2 and query half h = c%2 (1024 queries). Two algebraic rewrites kill
all inter-core communication:

  logits L = (x WQ)(x WK)^T / sqrt(D) = x M x^T   with M = WQ WK^T / sqrt(D)
  O = P (x WV) = (P x) WV = (x^T P)^T WV          (G^T := x^T P)

M is folded on the host (weight-only preprocessing, like conv/BN folding), so
the device needs no K projection and no K/V exchange: each core only needs its
batch's full x (shipped in both layouts, x^T for L and x for G^T) plus M and
WV. v3's three serialized pair AllGathers (~58us CC busy, ~20us PE stall
waiting on ccV2-fed data) disappear, as does the 8MB/core gather-reload DMA.
Per-core PE drops from 7 to 6 matmul units = 768 512-wide matmuls ~166us (the
K projection is the unit that M-folding deletes globally; 6 units/core is the
floor for this factorization -- no cross-core redundancy remains to shard).

SPMD uniformity: "own queries" are a h-dependent slice of the sequence, so the
host permutes each core's key/sequence order to own-half-first (softmax and
O are key-permutation invariant as long as x^T and x use the same permutation;
each core writes only its own query rows, un-permuted by the host gather).

Schedule per core (all bf16 operands, fp32 PSUM): memset constants + 160 PE
warmup mms right off the framework preamble (~7.1us); Q'^T = M^T x^T resident
(128 mm) pipelining with the input DMA, whose (m, xt) 512-col pieces land in
exact first-use order split across both rings -- only the qb=0 half runs up
front (group-0 strips need just Q' cols 0:512), with the qb=1 chains
deferred until after those strips so they run compute-paced once the B-half
DMAs have landed. The xs/wv loads ride the SYNC ring: load triggers occupy
the issuing engine queue ~600ns each, and 24 extra triggers on the scalar
queue measured as a 4.4us exp-then-PE stall once the strips start at ~27us.
Per q-group of 512: 16 L
strips (8 mm each) with exp -> bf16 P strips, the rowsum done as 4 free-dim-1
matmuls per strip (P slice stationary -> already-transposed [128,4] PSUM
accumulation; measured ~37ns each interleaved mid-chain -- a 16x216ns
full-width chain variant measured 6us SLOWER end-to-end, and GpSimd
CROSS_LANE_REDUCE is ~64us/strip, 150x too slow); G^T accumulated over the
16 P strips in two passes of 4 banks from the SHARED 7-buf PSUM pool that
also serves the Q' chains, L strips and O chains (identical [128,512]
write->read->free lifetime; one pool means no mid-kernel PSUM pool
transition, which measured as a ~0.8us barrier, and 7-deep rotation slack
everywhere; the 8th bank holds the rowsum accumulator); the kernel's very
last O chain is split into two 256-col sub-chains so its scale+store
pipeline under the second half. Output scaled by reciprocal rowsum straight
to bf16; mid-kernel stores all ride the sync ring (idle after the input
loads, so store triggers never jitter the scalar queue's exps), with only
the final split pair using both rings. fp8/DoubleRow was
measured numerically dead for this problem's 2e-2 gate (exp() amplifies
quant noise into multiplicative P error; every e4m3 insertion point alone
measures ~2e-2+ on the real inputs -- P/V/QK/xW, and even a Q'-phase-only
variant at the best M-scaling measured 2.8e-2), so everything stays bf16.

Verified on HW: 190.9us best / ~191-193us band across clean-clock runs (vs
261.9us v3 baseline; shared-device throttling occasionally inflates samples
to 198-229us with nonzero throttle counters), rel_err 4.29e-3 (gate 2e-2),
matching the numpy simulation of the pipeline exactly. PE busy ~176us of
the ~186.4us first-to-last-matmul span; startup is at the per-core HBM
bandwidth wall (~2MB gating set over 2 rings from the 7.3us ring start),
and the rest is the fixed ~7.4us framework preamble, ~4.7us tail/teardown,
and ~2.2us of metronomic ~432ns PE stalls every ~10.8us (50 matmuls) of
unknown, likely instruction-path, origin.
"""

import numpy as np
from contextlib import ExitStack

import ml_dtypes

import concourse.tile as tile
from concourse import bacc, mybir
from concourse.bass_utils import run_bass_kernel_spmd

F32 = mybir.dt.float32
BF16 = mybir.dt.bfloat16
EXP = mybir.ActivationFunctionType.Exp

B, S, D = 4, 2048, 1024
NQ = 1024          # query rows per core (own half, permuted-first)
QG = 512           # q-group width for the attention passes
NGROUPS = NQ // QG
NET = D // 128     # 8 e'-tiles (Q' feature dim)
NDT = D // 128     # 8 d-tiles (x feature dim)
NKT = S // 128     # 16 key/sequence tiles

_CACHE = {}


def _build_nc():
    nc = bacc.Bacc("TRN2", target_bir_lowering=False, debug=False)

    xt_d = nc.dram_tensor("xt", [D, S], BF16, kind="ExternalInput")
    xs_d = nc.dram_tensor("xs", [S, D], BF16, kind="ExternalInput")
    m_d = nc.dram_tensor("m", [D, D], BF16, kind="ExternalInput")
    wv_d = nc.dram_tensor("wv", [D, D], BF16, kind="ExternalInput")
    o_d = nc.dram_tensor("o", [NQ, D], BF16, kind="ExternalOutput")

    def dslc(i):
        return slice(i * 128, (i + 1) * 128)

    with tile.TileContext(nc) as tc, ExitStack() as ctx:
        # Constants come from on-device memsets (no DMA: the rings' head
        # slots stay free for the m/xt loads, and the PE warmup can start
        # right after the framework preamble instead of waiting on a DMA).
        small = ctx.enter_context(tc.tile_pool(name="small", bufs=1))
        ones16 = small.tile([128, 2], BF16, name="ones16", tag="ones16")
        nc.vector.memset(ones16[:], 1.0)
        ones32 = small.tile([1, 2], F32, name="ones32", tag="ones32")
        nc.vector.memset(ones32[:], 1.0)
        exp_warm = small.tile([1, 2], F32, name="exp_warm", tag="exp_warm")
        nc.scalar.activation(exp_warm[:], ones32[:], EXP, bias=0.0, scale=1.0)

        # M is only needed for the Q' phase: right stack, released before
        # the attention pools are allocated.
        mres = tc.alloc_tile_pool(name="mres", bufs=1, side="right")
        m_sb = [mres.tile([128, D], BF16, name=f"m{dt}", tag=f"m{dt}")
                for dt in range(NDT)]

        # Long-lived residents.
        xtres = ctx.enter_context(tc.tile_pool(name="xtres", bufs=1))
        xt_sb = [xtres.tile([128, S], BF16, name=f"xt{et}", tag=f"xt{et}")
                 for et in range(NET)]
        xsres = ctx.enter_context(tc.tile_pool(name="xsres", bufs=1))
        xs_sb = [xsres.tile([128, D], BF16, name=f"xs{st}", tag=f"xs{st}")
                 for st in range(NKT)]
        qres = ctx.enter_context(tc.tile_pool(name="qres", bufs=1))
        qt_sb = [qres.tile([128, NQ], BF16, name=f"qt{et}", tag=f"qt{et}")
                 for et in range(NET)]
        wvres = ctx.enter_context(tc.tile_pool(name="wvres", bufs=1))
        wv_sb = [wvres.tile([128, D], BF16, name=f"wv{dt}", tag=f"wv{dt}")
                 for dt in range(NDT)]

        # Input DMA split across both rings, ordered by first use. The first
        # Q' chain consumes (m[dt] cols 0:512, xt[dt] cols 0:512) for dt
        # 0..7 in order, so those land as interleaved (m, xt) pairs -- dt
        # 0-3 on the sync ring, dt 4-7 on scalar -- letting the chain's
        # matmuls pipeline with the DMA arrivals instead of waiting for a
        # bulk load. Then the B halves (m cols 512:1024 for the et>=4
        # chains, xt cols 512:1024 for qb=1), then the other-sequence-half
        # xt columns (first used at strip kt=8, ~55us), then xs (G pass,
        # ~70us) and wv (O chains, ~85us). Per-DMA ring occupancy is
        # ~600ns nearly independent of size in this regime, and a
        # full-width [128,1024]-piece variant measured 1.6us slower, so
        # the 512-col granularity stays.
        # 512-col pieces in first-use order; a full-width [128,1024]
        # xt-own variant measured 4.7us slower (completion granularity
        # outweighs DMA-count savings). xs/wv ride the SYNC ring: load
        # triggers occupy the issuing queue ~600ns each and extra triggers
        # on the scalar queue measured as a 4.4us exp (and thus PE) stall
        # once the group-0 strips start at ~27us.
        for dt in range(4):
            nc.sync.dma_start(m_sb[dt][:, 0:512], m_d.ap()[dslc(dt), 0:512])
            nc.sync.dma_start(xt_sb[dt][:, 0:512], xt_d.ap()[dslc(dt), 0:512])
        for dt in range(4, NDT):
            nc.scalar.dma_start(m_sb[dt][:, 0:512], m_d.ap()[dslc(dt), 0:512])
            nc.scalar.dma_start(xt_sb[dt][:, 0:512],
                                xt_d.ap()[dslc(dt), 0:512])
        for dt in range(4):
            nc.sync.dma_start(m_sb[dt][:, 512:1024],
                              m_d.ap()[dslc(dt), 512:1024])
        for dt in range(4, NDT):
            nc.scalar.dma_start(m_sb[dt][:, 512:1024],
                                m_d.ap()[dslc(dt), 512:1024])
        for dt in range(4):
            nc.sync.dma_start(xt_sb[dt][:, 512:1024],
                              xt_d.ap()[dslc(dt), 512:1024])
        for dt in range(4, NDT):
            nc.scalar.dma_start(xt_sb[dt][:, 512:1024],
                                xt_d.ap()[dslc(dt), 512:1024])
        for dt in range(NDT):
            nc.sync.dma_start(xt_sb[dt][:, 1024:2048],
                              xt_d.ap()[dslc(dt), 1024:2048])
        for st in range(NKT):
            nc.sync.dma_start(xs_sb[st][:], xs_d.ap()[dslc(st), :])
        for dt in range(NDT):
            nc.sync.dma_start(wv_sb[dt][:], wv_d.ap()[dslc(dt), :])

        # One shared 7-buf PSUM pool for every [128,512] accumulator: Q'
        # chains, L strips, G chains, and O chains all have the same
        # sequential write -> read -> free lifetime, so a single-tag
        # rotation gives every phase 7 banks of lookahead and there are NO
        # mid-kernel pool transitions (a released-then-reallocated PSUM
        # pool was measured to insert a ~0.8us barrier at the phase
        # boundary). The 8th bank holds the long-lived rowsum accumulator.
        bigps = ctx.enter_context(
            tc.tile_pool(name="bigps", bufs=7, space="PSUM"))
        rsps = ctx.enter_context(
            tc.tile_pool(name="rsps", bufs=1, space="PSUM"))

        # PE warmup during the input DMA (keeps HAM ramped into the Q'
        # phase; each warm matmul is ~60ns). The warm target is just the
        # first rotation slot of the shared pool.
        warm_ps = bigps.tile([1, 2], F32, name="warm_ps", tag="ps")
        for _ in range(160):
            nc.tensor.matmul(warm_ps[:], ones16[:, 0:1], ones16[:, 0:2],
                             start=True, stop=True)

        # ---- Q' phase: Q'^T = M^T x^T resident (own queries = cols 0:NQ).
        # Only the qb=0 half runs up front: group-0 strips need just Q'
        # cols 0:512, so the qb=1 chains are deferred until after those
        # strips -- by then the m/xt B-half DMAs have landed with ~30us of
        # slack and the deferred chains run fully compute-paced instead of
        # stalling on the second DMA wave.
        def qprime_chains(qb):
            for et in range(NET):
                ps = bigps.tile([128, 512], F32, name="qp_ps", tag="ps")
                for dt in range(NDT):
                    nc.tensor.matmul(
                        ps[:],
                        m_sb[dt][:, et * 128:(et + 1) * 128],
                        xt_sb[dt][:, qb * 512:(qb + 1) * 512],
                        start=(dt == 0), stop=(dt == NDT - 1))
                nc.vector.tensor_copy(
                    qt_sb[et][:, qb * 512:(qb + 1) * 512], ps[:])

        qprime_chains(0)

        # ---- Attention per q-group: L strips -> exp -> G^T -> O ----
        with tc.tile_pool(name="attp", bufs=2) as attp, \
             tc.tile_pool(name="gsbp", bufs=2) as gsbp, \
             tc.tile_pool(name="rssb", bufs=2) as rssb, \
             tc.tile_pool(name="osbp", bufs=3) as osbp:

            for g in range(NGROUPS):
                qslc = slice(g * QG, (g + 1) * QG)

                # L strips: L[k, q] = sum_e' x^T[e', k] Q'^T[e', q].
                # Rowsum: per strip, 4 free-dim-1 matmuls with the P slice
                # as the STATIONARY operand (out[128q, 1] += P[:, qtl].T @
                # ones) accumulate the rowsum already transposed in a
                # [128, 4] PSUM tile -- no [1,512] accumulator bank, no
                # transpose pass. They lag TWO strips behind the exp and are
                # interleaved mid-chain so their ldweights hide under the
                # 216ns L matmuls. (GpSimd CROSS_LANE_REDUCE was measured at
                # ~64us per [128,512] strip -- 150x too slow to use.)
                rs_t_ps = rsps.tile([128, QG // 128], F32, name="rs_t_ps",
                                    tag="rs_t_ps")

                def rowsum_mm(src_kt, qtl):
                    # start only on the very first matmul: start=True zeroes
                    # the whole 2KB PSUM zero-region (all 4 columns), and
                    # each column's first touch then consumes the pending-
                    # zero marking. A per-column start would wipe sibling
                    # columns' already-accumulated strip-0 values.
                    nc.tensor.matmul(
                        rs_t_ps[:, qtl:qtl + 1],
                        pt_strip[src_kt][:, qtl * 128:(qtl + 1) * 128],
                        ones16[:, 0:1],
                        start=(src_kt == 0 and qtl == 0),
                        stop=(src_kt == NKT - 1 and qtl == QG // 128 - 1))

                pt_strip = []
                for kt in range(NKT):
                    ps = bigps.tile([128, QG], F32, name="st_ps", tag="ps")
                    for et in range(NET):
                        nc.tensor.matmul(
                            ps[:],
                            xt_sb[et][:, kt * 128:(kt + 1) * 128],
                            qt_sb[et][:, qslc],
                            start=(et == 0), stop=(et == NET - 1))
                        if 3 <= et <= 6 and kt >= 2:
                            rowsum_mm(kt - 2, et - 3)
                    pt = attp.tile([128, QG], BF16, name=f"pt{kt}",
                                   tag=f"pt{kt}")
                    nc.scalar.activation(pt[:], ps[:], EXP, bias=0.0,
                                         scale=1.0)
                    pt_strip.append(pt)
                for src_kt in (NKT - 2, NKT - 1):
                    for qtl in range(QG // 128):
                        rowsum_mm(src_kt, qtl)

                # Deferred qb=1 Q' chains: after group-0's strips, before
                # its G passes (their B-half inputs landed ~35us ago).
                if g == 0:
                    qprime_chains(1)
                    mres.release()

                # G^T pass A (d-tiles 0-3): G^T[d, q] = sum_k x[k, d] P[k, q]
                g_sb = [None] * NDT

                def g_chain(dt):
                    gps = bigps.tile([128, QG], F32, name="gt_ps", tag="ps")
                    for kt in range(NKT):
                        nc.tensor.matmul(
                            gps[:],
                            xs_sb[kt][:, dt * 128:(dt + 1) * 128],
                            pt_strip[kt][:],
                            start=(kt == 0), stop=(kt == NKT - 1))
                    g_sb[dt] = gsbp.tile([128, QG], BF16, name=f"gt{dt}",
                                         tag=f"gt{dt}")
                    nc.vector.tensor_copy(g_sb[dt][:], gps[:])

                for dt in range(4):
                    g_chain(dt)

                # Rowsum finalize: the transposed [128, 4] accumulation only
                # needs a DVE reciprocal (first O-chain consumer is ~16us
                # out, well past the last rowsum matmul).
                rs_sb = rssb.tile([128, QG // 128], F32, name="rs_sb",
                                  tag="rs_sb")
                for qtl in range(QG // 128):
                    nc.vector.reciprocal(rs_sb[:, qtl:qtl + 1],
                                         rs_t_ps[:, qtl:qtl + 1])

                # G^T pass B (d-tiles 4-7) reuses the same 4 PSUM bufs; the
                # pass-A casts complete long before rotation needs them.
                for dt in range(4, NDT):
                    g_chain(dt)

                # O chains: O[q, e] = sum_d G^T[d, q]^T WV[d, e], scaled by
                # the reciprocal rowsum, stored bf16 on alternating rings.
                # The very last chain of the kernel is split into two
                # 256-col sub-chains so its scale+store pipeline under the
                # second half (and the final stores ride both rings),
                # trimming the exposed tail after the last matmul.
                for qtl in range(QG // 128):
                    for eb in range(2):
                        last = (g == NGROUPS - 1 and qtl == QG // 128 - 1
                                and eb == 1)
                        ebs = [(eb * 512, 256), (eb * 512 + 256, 256)] \
                            if last else [(eb * 512, 512)]
                        for ci, (c0, cw) in enumerate(ebs):
                            ops = bigps.tile([128, QG], F32, name="o_ps",
                                            tag="ps")
                            for dt in range(NDT):
                                nc.tensor.matmul(
                                    ops[:, 0:cw],
                                    g_sb[dt][:, qtl * 128:(qtl + 1) * 128],
                                    wv_sb[dt][:, c0:c0 + cw],
                                    start=(dt == 0), stop=(dt == NDT - 1))
                            osb = osbp.tile([128, 512], BF16, name="o_sb",
                                            tag="o_sb")
                            nc.vector.tensor_scalar_mul(
                                osb[:, 0:cw], ops[:, 0:cw],
                                rs_sb[:, qtl:qtl + 1])
                            # All mid-kernel stores ride the sync ring (idle
                            # after the input loads) so store triggers never
                            # jitter the scalar queue's exps; only the final
                            # split pair uses both rings for the tail.
                            eng = nc.scalar if (last and ci == 1) else nc.sync
                            eng.dma_start(
                                o_d.ap()[g * QG + qtl * 128:
                                         g * QG + (qtl + 1) * 128,
                                         c0:c0 + cw],
                                osb[:, 0:cw])

    nc.compile()
    return nc


def get_nc():
    if "nc" not in _CACHE:
        _CACHE["nc"] = _build_nc()
    return _CACHE["nc"]


def make_in_maps(x, WQ, WK, WV):
    bf16 = ml_dtypes.bfloat16
    # Weight folding on host (fp64 for accuracy): M = WQ WK^T / sqrt(D).
    M = (np.asarray(WQ, np.float64) @ np.asarray(WK, np.float64).T
         / np.sqrt(float(D)))
    m16 = np.ascontiguousarray(M.astype(np.float32).astype(bf16))
    wv16 = np.ascontiguousarray(np.asarray(WV, np.float32).astype(bf16))
    in_maps = []
    for c in range(8):
        b, h = c // 2, c % 2
        xb = np.asarray(x[b], np.float32)
        # Own-half-first sequence permutation keeps the SPMD program uniform.
        xp = np.concatenate(
            [xb[h * NQ:(h + 1) * NQ], xb[(1 - h) * NQ:(2 - h) * NQ]], axis=0)
        xp16 = xp.astype(bf16)
        in_maps.append({"xt": np.ascontiguousarray(xp16.T),
                        "xs": np.ascontiguousarray(xp16),
                        "m": m16, "wv": wv16})
    return in_maps


def kernel(**inputs):
    x = np.asarray(inputs["x"], dtype=np.float32)
    WQ = np.asarray(inputs["WQ"], dtype=np.float32)
    WK = np.asarray(inputs["WK"], dtype=np.float32)
    WV = np.asarray(inputs["WV"], dtype=np.float32)

    nc = get_nc()
    in_maps = make_in_maps(x, WQ, WK, WV)
    res = run_bass_kernel_spmd(nc, in_maps, core_ids=list(range(8)))

    out = np.empty((B, S, D), np.float32)
    for c in range(8):
        b, h = c // 2, c % 2
        out[b, h * NQ:(h + 1) * NQ, :] = np.asarray(
            res.results[c]["o"], dtype=np.float32)
    return out


if __name__ == "__main__":
    rng = np.random.default_rng(0)
    x = rng.standard_normal((B, S, D), dtype=np.float32)
    WQ = (rng.standard_normal((D, D), dtype=np.float32) * 0.02)
    WK = (rng.standard_normal((D, D), dtype=np.float32) * 0.02)
    WV = (rng.standard_normal((D, D), dtype=np.float32) * 0.02)
    o = kernel(x=x, WQ=WQ, WK=WK, WV=WV)
    print("out", o.shape, o.dtype, float(np.abs(o).max()))


# revision 45
# speedup vs baseline: 1.0263x; 1.0185x over previous
"""Single-head self-attention (B=4, S=2048, D=1024) on 8 Trainium2 NeuronCores.

v5: collective-free via weight folding + reassociation. Core c handles batch
b = c//2 and query half h = c%2 (1024 queries). Two algebraic rewrites kill
all inter-core communication:

  logits L = (x WQ)(x WK)^T / sqrt(D) = x M x^T   with M = WQ WK^T / sqrt(D)
  O = P (x WV) = (P x) WV = (x^T P)^T WV          (G^T := x^T P)

M is folded on the host (weight-only preprocessing, like conv/BN folding), so
the device needs no K projection and no K/V exchange: each core only needs its
batch's full x (shipped in both layouts, x^T for L and x for G^T) plus M and
WV. v3's three serialized pair AllGathers (~58us CC busy, ~20us PE stall
waiting on ccV2-fed data) disappear, as does the 8MB/core gather-reload DMA.
Per-core PE drops from 7 to 6 matmul units = 768 512-wide matmuls ~166us (the
K projection is the unit that M-folding deletes globally; 6 units/core is the
floor for this factorization -- no cross-core redundancy remains to shard).

SPMD uniformity: "own queries" are a h-dependent slice of the sequence, so the
host permutes each core's key/sequence order to own-half-first (softmax and
O are key-permutation invariant as long as x^T and x use the same permutation;
each core writes only its own query rows, un-permuted by the host gather).

Schedule per core (all bf16 operands, fp32 PSUM): memset constants + 160 PE
warmup mms right off the framework preamble (~7.1us); Q'^T = M^T x^T resident
(128 mm) pipelining with the input DMA, whose (m, xt) 512-col pieces land in
exact first-use order split across both rings -- only the qb=0 half runs up
front (group-0 strips need just Q' cols 0:512), with the qb=1 chains
deferred until after those strips so they run compute-paced once the B-half
DMAs have landed. The xs/wv loads ride the SYNC ring: load triggers occupy
the issuing engine queue ~600ns each, and 24 extra triggers on the scalar
queue measured as a 4.4us exp-then-PE stall once the strips start at ~27us.
Per q-group of 512: 16 L
strips (8 mm each) with exp -> bf16 P strips, the rowsum done as 4 free-dim-1
matmuls per strip (P slice stationary -> already-transposed [128,4] PSUM
accumulation; measured ~37ns each interleaved mid-chain -- a 16x216ns
full-width chain variant measured 6us SLOWER end-to-end, and GpSimd
CROSS_LANE_REDUCE is ~64us/strip, 150x too slow); G^T accumulated over the
16 P strips in two passes of 4 banks from the SHARED 7-buf PSUM pool that
also serves the Q' chains, L strips and O chains (identical [128,512]
write->read->free lifetime; one pool means no mid-kernel PSUM pool
transition, which measured as a ~0.8us barrier, and 7-deep rotation slack
everywhere; the 8th bank holds the rowsum accumulator); the kernel's very
last O chain is split into two 256-col sub-chains so its scale+store
pipeline under the second half. Output scaled by reciprocal rowsum straight
to bf16; mid-kernel stores all ride the sync ring (idle after the input
loads, so store triggers never jitter the scalar queue's exps), with only
the final split pair using both rings. fp8/DoubleRow was
measured numerically dead for this problem's 2e-2 gate (exp() amplifies
quant noise into multiplicative P error; every e4m3 insertion point alone
measures ~2e-2+ on the real inputs -- P/V/QK/xW, and even a Q'-phase-only
variant at the best M-scaling measured 2.8e-2), so everything stays bf16.

Verified on HW: 190.9us best / ~191-193us band across clean-clock runs (vs
261.9us v3 baseline; shared-device throttling occasionally inflates samples
to 198-229us with nonzero throttle counters), rel_err 4.29e-3 (gate 2e-2),
matching the numpy simulation of the pipeline exactly. PE busy ~176us of
the ~186.4us first-to-last-matmul span; startup is at the per-core HBM
bandwidth wall (~2MB gating set over 2 rings from the 7.3us ring start),
and the rest is the fixed ~7.4us framework preamble, ~4.7us tail/teardown,
and ~2.2us of metronomic ~432ns PE stalls every ~10.8us (50 matmuls) of
unknown, likely instruction-path, origin.
"""

import numpy as np
from contextlib import ExitStack

import ml_dtypes

import concourse.tile as tile
from concourse import bacc, mybir
from concourse.bass_utils import run_bass_kernel_spmd

F32 = mybir.dt.float32
BF16 = mybir.dt.bfloat16
EXP = mybir.ActivationFunctionType.Exp

B, S, D = 4, 2048, 1024
NQ = 1024          # query rows per core (own half, permuted-first)
QG = 512           # q-group width for the attention passes
NGROUPS = NQ // QG
NET = D // 128     # 8 e'-tiles (Q' feature dim)
NDT = D // 128     # 8 d-tiles (x feature dim)
NKT = S // 128     # 16 key/sequence tiles

_CACHE = {}


def _build_nc():
    nc = bacc.Bacc("TRN2", target_bir_lowering=False, debug=False)

    xt_d = nc.dram_tensor("xt", [D, S], BF16, kind="ExternalInput")
    xs_d = nc.dram_tensor("xs", [S, D], BF16, kind="ExternalInput")
    m_d = nc.dram_tensor("m", [D, D], BF16, kind="ExternalInput")
    wv_d = nc.dram_tensor("wv", [D, D], BF16, kind="ExternalInput")
    o_d = nc.dram_tensor("o", [NQ, D], BF16, kind="ExternalOutput")

    def dslc(i):
        return slice(i * 128, (i + 1) * 128)

    with tile.TileContext(nc) as tc, ExitStack() as ctx:
        # Constants come from on-device memsets (no DMA: the rings' head
        # slots stay free for the m/xt loads, and the PE warmup can start
        # right after the framework preamble instead of waiting on a DMA).
        small = ctx.enter_context(tc.tile_pool(name="small", bufs=1))
        ones16 = small.tile([128, 2], BF16, name="ones16", tag="ones16")
        nc.vector.memset(ones16[:], 1.0)
        ones32 = small.tile([1, 2], F32, name="ones32", tag="ones32")
        nc.vector.memset(ones32[:], 1.0)

        # M is only needed for the Q' phase: right stack, released before
        # the attention pools are allocated.
        mres = tc.alloc_tile_pool(name="mres", bufs=1, side="right")
        m_sb = [mres.tile([128, D], BF16, name=f"m{dt}", tag=f"m{dt}")
                for dt in range(NDT)]

        # Long-lived residents.
        xtres = ctx.enter_context(tc.tile_pool(name="xtres", bufs=1))
        xt_sb = [xtres.tile([128, S], BF16, name=f"xt{et}", tag=f"xt{et}")
                 for et in range(NET)]
        xsres = ctx.enter_context(tc.tile_pool(name="xsres", bufs=1))
        xs_sb = [xsres.tile([128, D], BF16, name=f"xs{st}", tag=f"xs{st}")
                 for st in range(NKT)]
        qres = ctx.enter_context(tc.tile_pool(name="qres", bufs=1))
        qt_sb = [qres.tile([128, NQ], BF16, name=f"qt{et}", tag=f"qt{et}")
                 for et in range(NET)]
        wvres = ctx.enter_context(tc.tile_pool(name="wvres", bufs=1))
        wv_sb = [wvres.tile([128, D], BF16, name=f"wv{dt}", tag=f"wv{dt}")
                 for dt in range(NDT)]

        # Input DMA split across both rings, ordered by first use. The first
        # Q' chain consumes (m[dt] cols 0:512, xt[dt] cols 0:512) for dt
        # 0..7 in order, so those land as interleaved (m, xt) pairs -- dt
        # 0-3 on the sync ring, dt 4-7 on scalar -- letting the chain's
        # matmuls pipeline with the DMA arrivals instead of waiting for a
        # bulk load. Then the B halves (m cols 512:1024 for the et>=4
        # chains, xt cols 512:1024 for qb=1), then the other-sequence-half
        # xt columns (first used at strip kt=8, ~55us), then xs (G pass,
        # ~70us) and wv (O chains, ~85us). Per-DMA ring occupancy is
        # ~600ns nearly independent of size in this regime, and a
        # full-width [128,1024]-piece variant measured 1.6us slower, so
        # the 512-col granularity stays.
        # 512-col pieces in first-use order; a full-width [128,1024]
        # xt-own variant measured 4.7us slower (completion granularity
        # outweighs DMA-count savings). xs/wv ride the SYNC ring: load
        # triggers occupy the issuing queue ~600ns each and extra triggers
        # on the scalar queue measured as a 4.4us exp (and thus PE) stall
        # once the group-0 strips start at ~27us.
        for dt in range(4):
            nc.sync.dma_start(m_sb[dt][:, 0:512], m_d.ap()[dslc(dt), 0:512])
            nc.sync.dma_start(xt_sb[dt][:, 0:512], xt_d.ap()[dslc(dt), 0:512])
        for dt in range(4, NDT):
            nc.scalar.dma_start(m_sb[dt][:, 0:512], m_d.ap()[dslc(dt), 0:512])
            nc.scalar.dma_start(xt_sb[dt][:, 0:512],
                                xt_d.ap()[dslc(dt), 0:512])
        for dt in range(4):
            nc.sync.dma_start(m_sb[dt][:, 512:1024],
                              m_d.ap()[dslc(dt), 512:1024])
        for dt in range(4, NDT):
            nc.scalar.dma_start(m_sb[dt][:, 512:1024],
                                m_d.ap()[dslc(dt), 512:1024])
        for dt in range(4):
            nc.sync.dma_start(xt_sb[dt][:, 512:1024],
                              xt_d.ap()[dslc(dt), 512:1024])
        for dt in range(4, NDT):
            nc.scalar.dma_start(xt_sb[dt][:, 512:1024],
                                xt_d.ap()[dslc(dt), 512:1024])
        for dt in range(NDT):
            nc.sync.dma_start(xt_sb[dt][:, 1024:2048],
                              xt_d.ap()[dslc(dt), 1024:2048])
        for st in range(NKT):
            nc.sync.dma_start(xs_sb[st][:], xs_d.ap()[dslc(st), :])
        for dt in range(NDT):
            nc.sync.dma_start(wv_sb[dt][:], wv_d.ap()[dslc(dt), :])

        # exp/ACT-table warm AFTER the scalar ring's load triggers: the
        # 1.28us ACT_TABLE_LOAD at the queue head was delaying the scalar
        # ring's first DMA by ~0.8us. Still ~20us before the first real exp.
        exp_warm = small.tile([1, 2], F32, name="exp_warm", tag="exp_warm")
        nc.scalar.activation(exp_warm[:], ones32[:], EXP, bias=0.0, scale=1.0)

        # One shared 7-buf PSUM pool for every [128,512] accumulator: Q'
        # chains, L strips, G chains, and O chains all have the same
        # sequential write -> read -> free lifetime, so a single-tag
        # rotation gives every phase 7 banks of lookahead and there are NO
        # mid-kernel pool transitions (a released-then-reallocated PSUM
        # pool was measured to insert a ~0.8us barrier at the phase
        # boundary). The 8th bank holds the long-lived rowsum accumulator.
        bigps = ctx.enter_context(
            tc.tile_pool(name="bigps", bufs=7, space="PSUM"))
        rsps = ctx.enter_context(
            tc.tile_pool(name="rsps", bufs=1, space="PSUM"))

        # PE warmup during the input DMA (keeps HAM ramped into the Q'
        # phase; each warm matmul is ~60ns). The warm target is just the
        # first rotation slot of the shared pool.
        warm_ps = bigps.tile([1, 2], F32, name="warm_ps", tag="ps")
        for _ in range(160):
            nc.tensor.matmul(warm_ps[:], ones16[:, 0:1], ones16[:, 0:2],
                             start=True, stop=True)

        # ---- Q' phase: Q'^T = M^T x^T resident (own queries = cols 0:NQ).
        # Only the qb=0 half runs up front: group-0 strips need just Q'
        # cols 0:512, so the qb=1 chains are deferred until after those
        # strips -- by then the m/xt B-half DMAs have landed with ~30us of
        # slack and the deferred chains run fully compute-paced instead of
        # stalling on the second DMA wave.
        def qprime_chains(qb):
            for et in range(NET):
                ps = bigps.tile([128, 512], F32, name="qp_ps", tag="ps")
                for dt in range(NDT):
                    nc.tensor.matmul(
                        ps[:],
                        m_sb[dt][:, et * 128:(et + 1) * 128],
                        xt_sb[dt][:, qb * 512:(qb + 1) * 512],
                        start=(dt == 0), stop=(dt == NDT - 1))
                nc.vector.tensor_copy(
                    qt_sb[et][:, qb * 512:(qb + 1) * 512], ps[:])

        qprime_chains(0)

        # ---- Attention per q-group: L strips -> exp -> G^T -> O ----
        with tc.tile_pool(name="attp", bufs=2) as attp, \
             tc.tile_pool(name="gsbp", bufs=2) as gsbp, \
             tc.tile_pool(name="rssb", bufs=2) as rssb, \
             tc.tile_pool(name="osbp", bufs=3) as osbp:

            for g in range(NGROUPS):
                qslc = slice(g * QG, (g + 1) * QG)

                # L strips: L[k, q] = sum_e' x^T[e', k] Q'^T[e', q].
                # Rowsum: per strip, 4 free-dim-1 matmuls with the P slice
                # as the STATIONARY operand (out[128q, 1] += P[:, qtl].T @
                # ones) accumulate the rowsum already transposed in a
                # [128, 4] PSUM tile -- no [1,512] accumulator bank, no
                # transpose pass. They lag TWO strips behind the exp and are
                # interleaved mid-chain so their ldweights hide under the
                # 216ns L matmuls. (GpSimd CROSS_LANE_REDUCE was measured at
                # ~64us per [128,512] strip -- 150x too slow to use.)
                rs_t_ps = rsps.tile([128, QG // 128], F32, name="rs_t_ps",
                                    tag="rs_t_ps")

                def rowsum_mm(src_kt, qtl):
                    # start only on the very first matmul: start=True zeroes
                    # the whole 2KB PSUM zero-region (all 4 columns), and
                    # each column's first touch then consumes the pending-
                    # zero marking. A per-column start would wipe sibling
                    # columns' already-accumulated strip-0 values.
                    nc.tensor.matmul(
                        rs_t_ps[:, qtl:qtl + 1],
                        pt_strip[src_kt][:, qtl * 128:(qtl + 1) * 128],
                        ones16[:, 0:1],
                        start=(src_kt == 0 and qtl == 0),
                        stop=(src_kt == NKT - 1 and qtl == QG // 128 - 1))

                pt_strip = []
                for kt in range(NKT):
                    ps = bigps.tile([128, QG], F32, name="st_ps", tag="ps")
                    for et in range(NET):
                        nc.tensor.matmul(
                            ps[:],
                            xt_sb[et][:, kt * 128:(kt + 1) * 128],
                            qt_sb[et][:, qslc],
                            start=(et == 0), stop=(et == NET - 1))
                        if 3 <= et <= 6 and kt >= 2:
                            rowsum_mm(kt - 2, et - 3)
                    pt = attp.tile([128, QG], BF16, name=f"pt{kt}",
                                   tag=f"pt{kt}")
                    nc.scalar.activation(pt[:], ps[:], EXP, bias=0.0,
                                         scale=1.0)
                    pt_strip.append(pt)
                for src_kt in (NKT - 2, NKT - 1):
                    for qtl in range(QG // 128):
                        rowsum_mm(src_kt, qtl)

                # Deferred qb=1 Q' chains: after group-0's strips, before
                # its G passes (their B-half inputs landed ~35us ago).
                if g == 0:
                    qprime_chains(1)
                    mres.release()

                # G^T pass A (d-tiles 0-3): G^T[d, q] = sum_k x[k, d] P[k, q]
                g_sb = [None] * NDT

                def g_chain(dt):
                    gps = bigps.tile([128, QG], F32, name="gt_ps", tag="ps")
                    for kt in range(NKT):
                        nc.tensor.matmul(
                            gps[:],
                            xs_sb[kt][:, dt * 128:(dt + 1) * 128],
                            pt_strip[kt][:],
                            start=(kt == 0), stop=(kt == NKT - 1))
                    g_sb[dt] = gsbp.tile([128, QG], BF16, name=f"gt{dt}",
                                         tag=f"gt{dt}")
                    nc.vector.tensor_copy(g_sb[dt][:], gps[:])

                for dt in range(4):
                    g_chain(dt)

                # Rowsum finalize: the transposed [128, 4] accumulation only
                # needs a DVE reciprocal (first O-chain consumer is ~16us
                # out, well past the last rowsum matmul).
                rs_sb = rssb.tile([128, QG // 128], F32, name="rs_sb",
                                  tag="rs_sb")
                for qtl in range(QG // 128):
                    nc.vector.reciprocal(rs_sb[:, qtl:qtl + 1],
                                         rs_t_ps[:, qtl:qtl + 1])

                # G^T pass B (d-tiles 4-7) reuses the same 4 PSUM bufs; the
                # pass-A casts complete long before rotation needs them.
                for dt in range(4, NDT):
                    g_chain(dt)

                # O chains: O[q, e] = sum_d G^T[d, q]^T WV[d, e], scaled by
                # the reciprocal rowsum, stored bf16 on alternating rings.
                # The very last chain of the kernel is split into two
                # 256-col sub-chains so its scale+store pipeline under the
                # second half (and the final stores ride both rings),
                # trimming the exposed tail after the last matmul.
                for qtl in range(QG // 128):
                    for eb in range(2):
                        last = (g == NGROUPS - 1 and qtl == QG // 128 - 1
                                and eb == 1)
                        ebs = [(eb * 512, 256), (eb * 512 + 256, 256)] \
                            if last else [(eb * 512, 512)]
                        for ci, (c0, cw) in enumerate(ebs):
                            ops = bigps.tile([128, QG], F32, name="o_ps",
                                            tag="ps")
                            for dt in range(NDT):
                                nc.tensor.matmul(
                                    ops[:, 0:cw],
                                    g_sb[dt][:, qtl * 128:(qtl + 1) * 128],
                                    wv_sb[dt][:, c0:c0 + cw],
                                    start=(dt == 0), stop=(dt == NDT - 1))
                            osb = osbp.tile([128, 512], BF16, name="o_sb",
                                            tag="o_sb")
                            nc.vector.tensor_scalar_mul(
                                osb[:, 0:cw], ops[:, 0:cw],
                                rs_sb[:, qtl:qtl + 1])
                            # All mid-kernel stores ride the sync ring (idle
                            # after the input loads) so store triggers never
                            # jitter the scalar queue's exps; only the final
                            # split pair uses both rings for the tail.
                            eng = nc.scalar if (last and ci == 1) else nc.sync
                            eng.dma_start(
                                o_d.ap()[g * QG + qtl * 128:
                                         g * QG + (qtl + 1) * 128,
                                         c0:c0 + cw],
                                osb[:, 0:cw])

    nc.compile()
    return nc


def get_nc():
    if "nc" not in _CACHE:
        _CACHE["nc"] = _build_nc()
    return _CACHE["nc"]


def make_in_maps(x, WQ, WK, WV):
    bf16 = ml_dtypes.bfloat16
    # Weight folding on host (fp64 for accuracy): M = WQ WK^T / sqrt(D).
    M = (np.asarray(WQ, np.float64) @ np.asarray(WK, np.float64).T
         / np.sqrt(float(D)))
    m16 = np.ascontiguousarray(M.astype(np.float32).astype(bf16))
    wv16 = np.ascontiguousarray(np.asarray(WV, np.float32).astype(bf16))
    in_maps = []
    for c in range(8):
        b, h = c // 2, c % 2
        xb = np.asarray(x[b], np.float32)
        # Own-half-first sequence permutation keeps the SPMD program uniform.
        xp = np.concatenate(
            [xb[h * NQ:(h + 1) * NQ], xb[(1 - h) * NQ:(2 - h) * NQ]], axis=0)
        xp16 = xp.astype(bf16)
        in_maps.append({"xt": np.ascontiguousarray(xp16.T),
                        "xs": np.ascontiguousarray(xp16),
                        "m": m16, "wv": wv16})
    return in_maps


def kernel(**inputs):
    x = np.asarray(inputs["x"], dtype=np.float32)
    WQ = np.asarray(inputs["WQ"], dtype=np.float32)
    WK = np.asarray(inputs["WK"], dtype=np.float32)
    WV = np.asarray(inputs["WV"], dtype=np.float32)

    nc = get_nc()
    in_maps = make_in_maps(x, WQ, WK, WV)
    res = run_bass_kernel_spmd(nc, in_maps, core_ids=list(range(8)))

    out = np.empty((B, S, D), np.float32)
    for c in range(8):
        b, h = c // 2, c % 2
        out[b, h * NQ:(h + 1) * NQ, :] = np.asarray(
            res.results[c]["o"], dtype=np.float32)
    return out


if __name__ == "__main__":
    rng = np.random.default_rng(0)
    x = rng.standard_normal((B, S, D), dtype=np.float32)
    WQ = (rng.standard_normal((D, D), dtype=np.float32) * 0.02)
    WK = (rng.standard_normal((D, D), dtype=np.float32) * 0.02)
    WV = (rng.standard_normal((D, D), dtype=np.float32) * 0.02)
    o = kernel(x=x, WQ=WQ, WK=WK, WV=WV)
    print("out", o.shape, o.dtype, float(np.abs(o).max()))
